# revision 38
# baseline (speedup 1.0000x reference)
"""Trainium2 Bass kernel for nn_AttentionBlock (B=4, N=1024, C=1024, H=16).

Sharding: 8 cores = 4 batches x 2 head-groups (8 heads each). Each core
computes its batch's tokens for its 8 heads end-to-end (fused qkv+delta
projection, qk-LayerNorm, RoPE, softmax attention with value-residual mix,
and a partial output projection over its head columns). The host sums the
two partial projections per batch.

Optimizations over the f32r baseline (452us -> ~260us):
- all matmul operands bf16 (same PE rate as f32r here, but half the DMA
  and SBUF footprint, cheaper LDWEIGHTS, faster packed DVE ops);
- xdT/w chunk DMAs alternate across the two hw queues so the q projection
  starts ~8us earlier and is never DMA-starved;
- ScalarE stays on one ACT table per phase (Square/Sqrt in phase A via
  rstd = 1/sqrt on DVE-reciprocal, Exp alone in attention) — no
  ACT_TABLE_LOAD thrash;
- LN/rope postprocess on packed bf16 SBUF tiles; each projection tile's
  only PSUM reader is a ScalarE copy, so PSUM banks free early and the
  k/v projections run t-outer with inline posts;
- rope uses host-expanded per-head cos / sign-folded sin tables (4 DVE
  ops, no rotate-half shuffles);
- softmax normalize: sums row -> SBUF, GpSimd partition_broadcast, DVE
  approx-reciprocal, one multiply (no DRAM roundtrip, nothing on PE/ACT);
- software-pipelined attention: paired score tiles (one 1024-wide exp per
  two key tiles), sc/exp emitted two chunks ahead of av, and PE idle
  slots filled with deferred v-projection tiles and, once a query half's
  finales land, that half's output-projection tiles;
- bf16 output staging/DMA (the host accumulates the two per-batch
  partial projections in f32 anyway).
"""
import os
import sys

sys.path.insert(0, "/opt/trn_rl_repo")

import numpy as np
import ml_dtypes

import concourse.bass as bass
import concourse.bacc as bacc
import concourse.tile as tile
from concourse import mybir
from concourse.bass_utils import run_bass_kernel_spmd
from concourse.masks import make_identity

F32 = mybir.dt.float32
F32R = mybir.dt.float32r
BF16 = mybir.dt.bfloat16

B, N, C, H = 4, 1024, 1024, 16
DH = C // H            # 64
HD = DH // 2           # 32
HPC = 8                # heads per core
NT = N // 128          # 8 token tiles
KC = (2 * C) // 128    # 16 contraction chunks for fused qkv+dt
EPS = 1e-5
AX = mybir.AxisListType.X
ALU = mybir.AluOpType
AF = mybir.ActivationFunctionType


def _bcast_free(ap, n, axis_pos=1):
    """Insert a step-0 free dim of size n at axis_pos of an AP."""
    new = list(ap.ap)
    new.insert(axis_pos, [0, n])
    return bass.AP(tensor=ap.tensor, offset=ap.offset, ap=new)


def _bcast_part(ap, n):
    """Partition-broadcast AP (step-0 partition dim) for DMA use."""
    return bass.AP(tensor=ap.tensor, offset=ap.offset, ap=[[0, n]] + list(ap.ap[1:]))


def build(lamb1, has_bias, has_ln, debug=False):
    """Build the single-core SPMD program.

    lamb1: python float (v-residual own-value weight; the residual weight
    lamb2 is folded into the host-prescaled vres input).
    has_bias: combined qkv+dt bias is nonzero -> biasd input present.
    has_ln: any qk-LayerNorm affine param nontrivial -> lnp input present.
    """
    nc = bacc.Bacc("TRN2", target_bir_lowering=False)

    xdT = nc.dram_tensor("xdT", [2 * C, N], BF16, kind="ExternalInput")
    w = nc.dram_tensor("w", [2 * C, 3 * HPC * DH], BF16, kind="ExternalInput")
    vres = nc.dram_tensor("vres", [N, HPC * DH], BF16, kind="ExternalInput")
    ropec = nc.dram_tensor("ropec", [N, HPC * DH], BF16, kind="ExternalInput")
    ropes = nc.dram_tensor("ropes", [N, HPC * DH], BF16, kind="ExternalInput")
    wproj = nc.dram_tensor("wproj", [HPC * DH, C], BF16, kind="ExternalInput")
    biasd = None
    if has_bias:
        biasd = nc.dram_tensor("biasd", [1, 3 * HPC * DH], F32, kind="ExternalInput")
    lnpd = None
    if has_ln:
        lnpd = nc.dram_tensor("lnp", [4, DH], BF16, kind="ExternalInput")
    out = nc.dram_tensor("out", [N, C], BF16, kind="ExternalOutput")
    dbg = {}
    if debug:
        for nm, shp, dt in [
                ("d_qr", [N, HPC * DH], BF16), ("d_kr", [N, HPC * DH], BF16),
                ("d_kT", [128, N], BF16), ("d_qT", [128, N], BF16),
                ("d_v", [N, HPC * (DH + 1)], BF16),
                ("d_ex", [128, 1024], BF16), ("d_av", [DH + 1, 512], F32),
                ("d_rcp", [1, 512], F32), ("d_rep", [DH, 512], F32),
                ("d_outT", [128, N], BF16)]:
            dbg[nm] = nc.dram_tensor(nm, shp, dt, kind="ExternalOutput")

    with tile.TileContext(nc) as tc:
        with (
            tc.tile_pool(name="const", bufs=1) as constp,
            tc.tile_pool(name="longp", bufs=1) as longp,
        ):
            ident = constp.tile([128, 128], BF16)
            make_identity(nc, ident)
            eps_t = constp.tile([128, 1], F32)
            nc.vector.memset(eps_t, EPS)

            bias_sb = None
            if biasd is not None:
                bias_sb = constp.tile([128, 3 * HPC * DH], F32)
                nc.scalar.dma_start(out=bias_sb, in_=_bcast_part(biasd[:, :], 128))
            ln_sb = None
            if lnpd is not None:
                ln_sb = constp.tile([128, 4, DH], BF16)
                nc.scalar.dma_start(out=ln_sb, in_=_bcast_part(lnpd[:, :], 128))

            # big persistent SBUF tensors
            xdT_sb = longp.tile([128, KC, N], BF16)
            w_sb = longp.tile([128, KC, 3 * HPC * DH], BF16)
            rpc_sb = longp.tile([128, NT, HPC, DH], BF16)
            rps_sb = longp.tile([128, NT, HPC, DH], BF16)
            vres_sb = longp.tile([128, NT, HPC, DH], BF16)
            v_sb = longp.tile([128, NT, HPC, DH + 1], BF16)
            qT_sb = longp.tile([128, HPC // 2, N], BF16)
            kT_sb = longp.tile([128, HPC // 2, N], BF16)
            outT_sb = longp.tile([128, HPC // 2, N], BF16)
            wproj_sb = longp.tile([128, 4, C], BF16)

            # ---- input DMAs: xdT/w chunk pairs alternate across the two
            # hw queues (balanced ~5.2MB each); per-DMA issue cost is ~700ns
            # on the issuing queue, so fewer, bigger DMAs win. Non-critical
            # tensors stream after.
            nc.sync.dma_start(out=xdT_sb[:, 0, 0:256],
                              in_=xdT[0:128, 0:256])
            nc.gpsimd.dma_start(out=w_sb[:, 0, 0:512], in_=w[0:128, 0:512])
            nc.sync.dma_start(out=xdT_sb[:, 0, 256:N], in_=xdT[0:128, 256:N])
            nc.gpsimd.dma_start(out=w_sb[:, 0, 512:1536],
                                in_=w[0:128, 512:1536])
            for kc in range(1, KC):
                qa, qb = ((nc.sync, nc.gpsimd) if kc % 2 == 0
                          else (nc.gpsimd, nc.sync))
                qa.dma_start(out=xdT_sb[:, kc, :],
                             in_=xdT[kc * 128:(kc + 1) * 128, :])
                qb.dma_start(out=w_sb[:, kc, :],
                             in_=w[kc * 128:(kc + 1) * 128, :])
            rr = "(t p) (h d) -> p t h d"
            nc.sync.dma_start(out=rpc_sb,
                              in_=ropec[:, :].rearrange(rr, p=128, h=HPC))
            nc.sync.dma_start(out=rps_sb,
                              in_=ropes[:, :].rearrange(rr, p=128, h=HPC))
            nc.gpsimd.dma_start(out=vres_sb,
                                in_=vres[:, :].rearrange(rr, p=128, h=HPC))
            for cc in range(4):
                nc.gpsimd.dma_start(out=wproj_sb[:, cc, :],
                                    in_=wproj[cc * 128:(cc + 1) * 128, :])

            # ones column of v (denominator row of the av matmul)
            nc.vector.memset(v_sb[:, :, :, DH:DH + 1], 1.0)

            # ------------- phase A: fused qkv+dt projection, LN, rope ------
            # q runs kc-outer (tracks DMA chunk arrival); k and v run t-outer
            # with inline postprocessing so PSUM tiles free progressively.
            # Each projection tile is first copied to bf16 SBUF by ScalarE —
            # that copy is the tile's only PSUM reader, so the bank frees
            # ~1us after the matmuls finish, and the LN/rope math runs on
            # fast packed-bf16 SBUF DVE ops.
            with (
                tc.tile_pool(name="qkp", bufs=1) as qkp,
                tc.tile_pool(name="psA", bufs=8, space="PSUM") as psA,
                tc.tile_pool(name="scrA", bufs=3) as scrA,
                tc.tile_pool(name="stat", bufs=4) as stat,
            ):
                qr_sb = qkp.tile([128, NT, HPC, DH], BF16)
                kr_sb = qkp.tile([128, NT, HPC, DH], BF16)

                def post_qk(ps, t, ob):
                    ps3 = ps.rearrange("p (h d) -> p h d", h=HPC)
                    if bias_sb is not None:
                        nc.vector.tensor_add(
                            ps[:], ps[:], bias_sb[:, ob * 512:(ob + 1) * 512])
                    xb = scrA.tile([128, HPC, DH], BF16, tag="xb")
                    nc.scalar.activation(out=xb[:], in_=ps3, func=AF.Copy)
                    red_s = stat.tile([128, HPC], F32, tag="red_s")
                    nc.vector.reduce_sum(out=red_s[:], in_=xb[:], axis=AX)
                    sqb = scrA.tile([128, HPC, DH], BF16, tag="sqb")
                    nc.vector.tensor_mul(sqb[:], xb[:], xb[:])
                    red_q = stat.tile([128, HPC], F32, tag="red_q")
                    nc.vector.reduce_sum(out=red_q[:], in_=sqb[:], axis=AX)
                    mean = stat.tile([128, HPC], F32, tag="mean")
                    nc.vector.tensor_scalar_mul(mean[:], in0=red_s[:],
                                                scalar1=1.0 / DH)
                    var = stat.tile([128, HPC], F32, tag="var")
                    nc.vector.tensor_mul(var[:], mean[:], mean[:])
                    nc.vector.scalar_tensor_tensor(
                        out=var[:], in0=red_q[:], scalar=1.0 / DH,
                        in1=var[:], op0=ALU.mult, op1=ALU.subtract)
                    # rstd = 1/sqrt(var+eps); Sqrt+Square share an ACT
                    # table set, the DVE reciprocal keeps Exp's table
                    # untouched for the attention phase.
                    rstd = stat.tile([128, HPC], F32, tag="rstd")
                    nc.scalar.activation(out=rstd[:], in_=var[:], func=AF.Sqrt,
                                         bias=eps_t[:])
                    nc.vector.reciprocal(rstd[:], rstd[:])
                    mr = stat.tile([128, HPC], F32, tag="mr")
                    nc.vector.tensor_mul(mr[:], mean[:], rstd[:])
                    # xr = xb*rstd - mean*rstd
                    xr = scrA.tile([128, HPC, DH], BF16, tag="xr")
                    nc.vector.scalar_tensor_tensor(
                        out=xr[:], in0=xb[:], scalar=1.0,
                        in1=_bcast_free(rstd[:], DH, 2)[:],
                        op0=ALU.mult, op1=ALU.mult)
                    nc.vector.tensor_tensor(
                        out=xr[:], in0=xr[:], in1=_bcast_free(mr[:], DH, 2)[:],
                        op=ALU.subtract)
                    if ln_sb is not None:
                        gi, bi = (0, 1) if ob == 0 else (2, 3)
                        nc.vector.tensor_tensor(
                            out=xr[:], in0=xr[:],
                            in1=_bcast_free(ln_sb[:, gi, :], HPC, 1)[:],
                            op=ALU.mult)
                        nc.vector.tensor_tensor(
                            out=xr[:], in0=xr[:],
                            in1=_bcast_free(ln_sb[:, bi, :], HPC, 1)[:],
                            op=ALU.add)
                    # rope: dst = xr*cos + rot_half(xr)*sin'
                    dst = (qr_sb if ob == 0 else kr_sb)
                    tc_ = scrA.tile([128, HPC, DH], BF16, tag="tc")
                    nc.vector.tensor_tensor(out=tc_[:], in0=xr[:],
                                            in1=rpc_sb[:, t], op=ALU.mult)
                    tm = scrA.tile([128, HPC, DH], BF16, tag="tm")
                    nc.vector.tensor_tensor(out=tm[:, :, 0:HD],
                                            in0=xr[:, :, HD:DH],
                                            in1=rps_sb[:, t, :, 0:HD],
                                            op=ALU.mult)
                    nc.vector.tensor_tensor(out=tm[:, :, HD:DH],
                                            in0=xr[:, :, 0:HD],
                                            in1=rps_sb[:, t, :, HD:DH],
                                            op=ALU.mult)
                    nc.vector.tensor_tensor(out=dst[:, t], in0=tc_[:],
                                            in1=tm[:], op=ALU.add)

                def post_v(ps, t, on_dve):
                    ps3 = ps.rearrange("p (h d) -> p h d", h=HPC)
                    if bias_sb is not None:
                        nc.vector.tensor_add(
                            ps[:], ps[:], bias_sb[:, 1024:1536])
                    if on_dve:
                        # attention-phase tiles: keep ScalarE free for exps
                        nc.vector.tensor_scalar_mul(
                            v_sb[:, t, :, 0:DH], in0=ps3, scalar1=float(lamb1))
                    else:
                        nc.scalar.activation(out=v_sb[:, t, :, 0:DH], in_=ps3,
                                             func=AF.Copy, scale=float(lamb1))
                    nc.vector.tensor_tensor(out=v_sb[:, t, :, 0:DH],
                                            in0=v_sb[:, t, :, 0:DH],
                                            in1=vres_sb[:, t], op=ALU.add)

                def proj_tile(ob, t, pool, tag):
                    ps = pool.tile([128, 512], F32, name=f"pt{ob}_{t}",
                                   tag=tag)
                    for kc in range(KC):
                        nc.tensor.matmul(
                            ps[:],
                            xdT_sb[:, kc, t * 128:(t + 1) * 128],
                            w_sb[:, kc, ob * 512:(ob + 1) * 512],
                            start=(kc == 0), stop=(kc == KC - 1))
                    return ps

                def transpose_ob(src, dstT):
                    for j in range(HPC // 2):
                        tp = psA.tile([128, 512], F32, tag="pp")
                        tpb = tp.bitcast(BF16)
                        for t in range(NT):
                            nc.tensor.transpose(
                                tpb[:, t * 128:(t + 1) * 128],
                                src[:, t, 2 * j:2 * j + 2, :]
                                   .rearrange("p h d -> p (h d)"),
                                ident[:])
                        if dstT is qT_sb:
                            nc.scalar.activation(out=dstT[:, j, :], in_=tpb[:],
                                                 func=AF.Copy)
                        else:
                            nc.vector.tensor_copy(dstT[:, j, :], tpb[:])

                # q: bulk kc-outer (tracks DMA chunk arrival), then a
                # t-outer tail with inline posts so the DVE postprocess
                # stream is spread instead of bursting 8 chains at once.
                KS = KC - 4
                ps_tiles = [psA.tile([128, 512], F32, name=f"pp{_t}",
                                     tag="pp")
                            for _t in range(NT)]
                for kc in range(KS):
                    for t in range(NT):
                        nc.tensor.matmul(
                            ps_tiles[t][:],
                            xdT_sb[:, kc, t * 128:(t + 1) * 128],
                            w_sb[:, kc, 0:512],
                            start=(kc == 0), stop=False)
                for t in range(NT):
                    for kc in range(KS, KC):
                        nc.tensor.matmul(
                            ps_tiles[t][:],
                            xdT_sb[:, kc, t * 128:(t + 1) * 128],
                            w_sb[:, kc, 0:512],
                            start=False, stop=(kc == KC - 1))
                    post_qk(ps_tiles[t], t, 0)
                # k, then both transposes, then the first half of v
                for t in range(NT):
                    post_qk(proj_tile(1, t, psA, "pp"), t, 1)
                # v tiles before the transposes: 15us of independent PE work
                # covers the DVE draining the k postprocess chains, so the
                # transposes (which need every k tile's rope done) run
                # gap-free right before attention consumes them.
                for t in range(4):
                    post_v(proj_tile(2, t, psA, "pp"), t, on_dve=False)
                transpose_ob(qr_sb, qT_sb)
                transpose_ob(kr_sb, kT_sb)
                if debug:
                    rr2 = "(t p) (h d) -> p t h d"
                    nc.sync.dma_start(
                        out=dbg["d_qr"][:, :].rearrange(rr2, p=128, h=HPC),
                        in_=qr_sb)
                    nc.sync.dma_start(
                        out=dbg["d_kr"][:, :].rearrange(rr2, p=128, h=HPC),
                        in_=kr_sb)
                    nc.sync.dma_start(out=dbg["d_qT"][:, :], in_=qT_sb[:, 0, :])
                    nc.sync.dma_start(out=dbg["d_kT"][:, :], in_=kT_sb[:, 0, :])

            # ------------- attention + interleaved fillers ------------------
            # qh-major iteration. PE filler between attention chunks: first
            # the deferred second half of the v projection (tiles 4-7), then,
            # once a query-half's 8 finales are done, that half's output
            # projection tiles. This keeps the PE streaming through the
            # exp-latency hops.
            with (
                tc.tile_pool(name="psS", bufs=2, space="PSUM") as psS,
                tc.tile_pool(name="psV", bufs=2, space="PSUM") as psV,
                tc.tile_pool(name="psP", bufs=2, space="PSUM") as psP,
                tc.tile_pool(name="expp", bufs=6) as expp,
                tc.tile_pool(name="nrm", bufs=2) as nrm,
                tc.tile_pool(name="outp", bufs=2) as outp,
            ):
                iters = [(j, hh, qh)
                         for qh in range(2)
                         for j in range(HPC // 2)
                         for hh in range(2)]
                NKK = NT // 2  # score-tile pairs per iteration
                av_t = {}
                scale = 1.0 / float(np.sqrt(DH))

                def emit_sc(it_idx, kk):
                    j, hh, qh = iters[it_idx]
                    ro = 64 * hh
                    sc = psS.tile([128, 2, 512], F32, tag="sc")
                    for i in range(2):
                        kc = 2 * kk + i
                        nc.tensor.matmul(
                            sc[:, i, :],
                            kT_sb[ro:ro + DH, j, kc * 128:(kc + 1) * 128],
                            qT_sb[ro:ro + DH, j, qh * 512:(qh + 1) * 512],
                            start=True, stop=True,
                            tile_position=(ro, 0))
                    ex = expp.tile([128, 2, 512], BF16, tag="ex")
                    nc.scalar.activation(out=ex[:], in_=sc[:], func=AF.Exp,
                                         scale=scale)
                    if debug and it_idx == 0 and kk == 0:
                        nc.sync.dma_start(
                            out=dbg["d_ex"][:, :].rearrange(
                                "p (a b) -> p a b", a=2),
                            in_=ex[:])
                    return ex

                def emit_av(it_idx, kk, ex):
                    j, hh, qh = iters[it_idx]
                    h = 2 * j + hh
                    if kk == 0:
                        av_t[it_idx] = psV.tile([DH + 1, 512], F32,
                                                name=f"av{it_idx}", tag="av")
                    av = av_t[it_idx]
                    for i in range(2):
                        kc = 2 * kk + i
                        nc.tensor.matmul(
                            av[:], v_sb[:, kc, h, :], ex[:, i, :],
                            start=(kc == 0), stop=(kc == NT - 1))

                def emit_finale(it_idx):
                    j, hh, qh = iters[it_idx]
                    ro = 64 * hh
                    av = av_t.pop(it_idx)
                    # sums row (psum partition 64) -> partition 0 SBUF, then
                    # gpsimd-broadcast to 64 partitions, then approx-recip
                    # there (the custom DVE op misbehaves at base >= 64).
                    sums = nrm.tile([1, 512], F32, tag="sums")
                    nc.scalar.activation(out=sums[:], in_=av[DH:DH + 1, :],
                                         func=AF.Copy)
                    rep = nrm.tile([DH, 2, 512], F32, tag="rep")
                    nc.gpsimd.partition_broadcast(
                        rep[:, 0, :], sums[:], channels=DH)
                    nc.vector.reciprocal_approx_fast(
                        out=rep[:, 1, :], in_=rep[:, 0, :])
                    nc.vector.tensor_tensor(
                        out=outT_sb[ro:ro + DH, j, qh * 512:(qh + 1) * 512],
                        in0=av[0:DH, :], in1=rep[:, 1, :], op=ALU.mult)
                    if debug and it_idx == 0:
                        avc = nrm.tile([DH + 1, 512], F32, tag="avc")
                        nc.vector.tensor_copy(avc[:], av[:])
                        nc.sync.dma_start(out=dbg["d_av"][:, :], in_=avc)
                        nc.sync.dma_start(out=dbg["d_rcp"][:, :],
                                          in_=rep[0:1, 1, :])
                        nc.sync.dma_start(out=dbg["d_rep"][:, :],
                                          in_=rep[:, 1, :])

                proj_units = [(t, oh) for t in range(NT) for oh in range(2)]
                stg_t = {}
                state = {"emitted": 0, "finales": 0, "vdef": 4}

                def emit_proj_unit():
                    t, oh = proj_units[state["emitted"]]
                    state["emitted"] += 1
                    if oh == 0:
                        stg_t[t] = outp.tile([128, C], BF16, name=f"stg{t}",
                                             tag="stg")
                    pp = psP.tile([128, 512], F32, tag="pp2")
                    for cc in range(4):
                        nc.tensor.matmul(
                            pp[:],
                            outT_sb[:, cc, t * 128:(t + 1) * 128],
                            wproj_sb[:, cc, oh * 512:(oh + 1) * 512],
                            start=(cc == 0), stop=(cc == 3))
                    nc.vector.tensor_copy(
                        stg_t[t][:, oh * 512:(oh + 1) * 512], pp[:])
                    if oh == 1:
                        q = nc.sync if t % 2 == 0 else nc.gpsimd
                        q.dma_start(out=out[t * 128:(t + 1) * 128, :],
                                    in_=stg_t.pop(t))

                def emit_filler():
                    if state["vdef"] < NT:
                        t = state["vdef"]
                        state["vdef"] += 1
                        post_v(proj_tile(2, t, psP, "pp2"), t, on_dve=True)
                        return
                    eligible = (state["finales"] // 8) * 8
                    if state["emitted"] < eligible:
                        emit_proj_unit()

                # pipeline: sc/exp run 2 chunks ahead of av; the finale (all
                # DVE/GpSimd) is emitted as soon as its last av lands.
                chunks = [(i, kk) for i in range(len(iters))
                          for kk in range(NKK)]
                exq = []      # (it_idx, kk, ex) awaiting av emission
                for (it_idx, kk) in chunks:
                    if len(exq) >= 3:
                        ai, akk, aex = exq.pop(0)
                        emit_av(ai, akk, aex)
                        if akk == NKK - 1:
                            emit_finale(ai)
                            state["finales"] += 1
                    ex = emit_sc(it_idx, kk)
                    exq.append((it_idx, kk, ex))
                    # NOTE: with vdef=3 every deferred v tile T is emitted at
                    # least one chunk before the first av matmul that reads
                    # it (t_T filler at chunk T-3, first read at chunk T//2+2
                    # > T-3 for T>=3), so reads always see the RAW write.
                    emit_filler()
                for (ai, akk, aex) in exq:
                    emit_av(ai, akk, aex)
                    if akk == NKK - 1:
                        emit_finale(ai)
                        state["finales"] += 1
                if debug:
                    nc.sync.dma_start(out=dbg["d_outT"][:, :],
                                      in_=outT_sb[:, 0, :])
                    nc.sync.dma_start(
                        out=dbg["d_v"][:, :].rearrange(
                            "(t p) (h d) -> p t h d", p=128, h=HPC),
                        in_=v_sb)
                while state["emitted"] < len(proj_units):
                    emit_proj_unit()

    nc.finalize()
    return nc


_CACHE = {}
_LAST_RES = None


def _bf16(a):
    return np.ascontiguousarray(a.astype(ml_dtypes.bfloat16))


def kernel(x, rope, delta_t_emb, v_residual_v1, Wqkv, bqkv, Wdt, bdt,
           qn_g, qn_b, kn_g, kn_b, lamb1, lamb2, Wproj, bproj):
    x = np.asarray(x, np.float32)
    rope = np.ascontiguousarray(np.asarray(rope, np.float32))
    delta_t_emb = np.asarray(delta_t_emb, np.float32)
    v_residual_v1 = np.asarray(v_residual_v1, np.float32)
    Wqkv = np.asarray(Wqkv, np.float32)
    Wdt = np.asarray(Wdt, np.float32)
    Wproj = np.asarray(Wproj, np.float32)
    bias = np.asarray(bqkv, np.float32) + np.asarray(bdt, np.float32)
    l1 = float(np.asarray(lamb1)); l2 = float(np.asarray(lamb2))
    qn_g = np.asarray(qn_g, np.float32); qn_b = np.asarray(qn_b, np.float32)
    kn_g = np.asarray(kn_g, np.float32); kn_b = np.asarray(kn_b, np.float32)

    has_bias = bool(np.any(bias))
    has_ln = not (np.all(qn_g == 1.0) and np.all(qn_b == 0.0)
                  and np.all(kn_g == 1.0) and np.all(kn_b == 0.0))

    dbgf = bool(int(os.environ.get("KERNEL_DEBUG", "0")))
    key = (l1, has_bias, has_ln, dbgf)
    if key not in _CACHE:
        _CACHE[key] = build(l1, has_bias, has_ln, debug=dbgf)
    nc = _CACHE[key]

    # host-prepared rope tables, expanded across the 8 heads per core:
    # cos table and sign-folded sin table (rotate_half absorbed:
    # out = x*cos + rot(x)*sin' with sin' = [-sin_lo || sin_hi]).
    sin = rope[:, 0:DH]; cos = rope[:, DH:2 * DH]
    sinp = np.concatenate([-sin[:, 0:HD], sin[:, HD:DH]], axis=1)
    cos_exp = _bf16(np.tile(cos[:, None, :], (1, HPC, 1)).reshape(N, HPC * DH))
    sin_exp = _bf16(np.tile(sinp[:, None, :], (1, HPC, 1)).reshape(N, HPC * DH))

    in_maps = []
    for c in range(8):
        b = c // 2
        g = c % 2
        rsl = slice(g * 512, (g + 1) * 512)
        w_core = np.concatenate([
            np.concatenate([Wqkv[rsl], Wqkv[C:][rsl], Wqkv[2 * C:][rsl]], 0).T,
            np.concatenate([Wdt[rsl], Wdt[C:][rsl], Wdt[2 * C:][rsl]], 0).T,
        ], axis=0)
        m = {
            "xdT": _bf16(np.concatenate([x[b].T, delta_t_emb[b].T], 0)),
            "w": _bf16(w_core),
            "vres": _bf16(l2 * v_residual_v1[b, g * 8:(g + 1) * 8]
                          .transpose(1, 0, 2).reshape(N, 512)),
            "ropec": cos_exp,
            "ropes": sin_exp,
            "wproj": _bf16(Wproj[:, rsl].T),
        }
        if has_bias:
            bc = np.concatenate([bias[rsl], bias[C:][rsl], bias[2 * C:][rsl]])
            m["biasd"] = np.ascontiguousarray(bc[None, :].astype(np.float32))
        if has_ln:
            m["lnp"] = _bf16(np.stack([qn_g, qn_b, kn_g, kn_b], 0))
        in_maps.append(m)

    trace = bool(int(os.environ.get("KERNEL_TRACE", "0")))
    res = run_bass_kernel_spmd(nc, in_maps, core_ids=list(range(8)), trace=trace)
    global _LAST_RES
    _LAST_RES = res
    if trace and res.exec_time_ns is not None:
        print(f"HW exec time: {res.exec_time_ns} ns")
        kernel.last_exec_time_ns = res.exec_time_ns
        kernel.last_results = res

    out = np.empty((B, N, C), np.float32)
    for b in range(B):
        out[b] = (res.results[2 * b]["out"].astype(np.float32)
                  + res.results[2 * b + 1]["out"].astype(np.float32))
    bproj = np.asarray(bproj, np.float32)
    if np.any(bproj):
        out += bproj[None, None, :]
    return out
